# revision 8
# baseline (speedup 1.0000x reference)
"""Trainium2 Bass kernel for nn_AdaptiveAlphaQuantizedLinear.

out[b,t,k] = sum_n x[b,t,n]*mu1[n] * ((W_q[k,n]-zeros[k,g(n)])*scales[k,g(n)])*mu2[k]
             + bias[k]

Strategy (8 NeuronCores, tensor-parallel along K), v7:
  Host prep:
    - Codes centered: T = 2*W_q-15 (odd ints, |T|<=15); a2[k,g] =
      scales*mu2/2 so W_deq = a2*T + a*(7.5-zeros).  The affine part
      rides the Xg/ones extra-contraction-row matmul (c-term).
    - x' = x*mu1 in bf16; contraction order INTERLEAVED (tile t,
      partition p -> n = (p//2)*128 + 2t + (p%2)) so the dequant scale
      tile srep[p,k] = a2[k, p//2] is one constant [128,KSH] tile.
    - Tiles grouped in blocks of 10: 4 PRE-DEQUANTIZED fp8 E3M4 tiles
      (stream STRAIGHT into the PE as the moving operand; mixed-dtype
      bf16-stationary x fp8e3-moving matmul is exact on TRN2, incl.
      subnormals - HW-verified) + 6 centered-int8-code tiles dequanted
      on DVE (x srep).  28 fp8 + 36 int8 tiles -> rel-err ~1.0e-2
      (budget 2e-2).  W DMA 8MB/core in 13 block transfers of 0.5/0.75MB
      (near-peak DMA efficiency, few SP-queue issues).
  Device per core:
    - 32 warm-up matmuls on a memset tile right after the preamble keep
      the PE HAM clock-gate busy during the head DMA wait, so real
      matmuls start at 2.4 GHz (saves ~4us of cold-clock).
    - W blocks on the SP HWDGE queue; xt/srep/ct/xg on ACT.  DVE
      dequants int8 tiles; PE runs 4 accumulating matmuls per tile.
    - Xg/ones rows close the accumulation with the affine term mid-stream.
    - Tail: PSUM evacuation split ACT/DVE immediately behind the final
      4 matmuls; one out DMA per bt-half on sync/scalar queues.
  host: concat k-shards, reshape to [8, 32, 8192].
"""
import sys
sys.path.insert(0, "/opt/trn_rl_repo")
import numpy as np

K = 8192
N = 8192
GROUP_SIZE = 128
NG = N // GROUP_SIZE          # 64 groups
B, T = 8, 32
BT = B * T                    # 256
NCORES = 8
KSH = K // NCORES             # 1024 out-features per core
NT = N // 128                 # 64 n-tiles
# 28 fp8-direct + 36 int8 tiles arranged so the head is dir-heavy (the
# PE can start before srep/DVE are up) and the first i8 run is short
# (DVE builds lookahead before the first long i8 stretch).  Runs are
# consecutive same-category groups; each run is one DMA.
RUNS = ([("dir", [0]), ("dir", [1, 2]), ("dir", [3, 4, 5]), ("dir", [6, 7]),
         ("i8", [8, 9]), ("dir", [10, 11, 12, 13]), ("i8", list(range(14, 20)))]
        + sum([[("dir", list(range(20 + 10 * j, 24 + 10 * j))),
                ("i8", list(range(24 + 10 * j, 30 + 10 * j)))]
               for j in range(4)], [])
        + [("i8", [60, 61, 62, 63])])
IS_DIR = [None] * NT
RUN_OF = {}
WOFF = {}                     # run -> column-tile offset into its dram tensor
_off = {"dir": 0, "i8": 0}
for r, (kind, tiles) in enumerate(RUNS):
    WOFF[r] = _off[kind]
    _off[kind] += len(tiles)
    for i, t in enumerate(tiles):
        RUN_OF[t] = (r, i)
        IS_DIR[t] = kind == "dir"
HT = sum(IS_DIR)              # 28
WARM = 33                     # PE warm-up matmuls (~3.5us at 1.2GHz)

_NC_CACHE = None


def _build():
    from concourse import bacc, tile, mybir

    bf16 = mybir.dt.bfloat16
    f8e3 = mybir.dt.float8e3
    nc = bacc.Bacc("TRN2", target_bir_lowering=False, debug=False,
                   num_devices=NCORES)
    wdir = nc.dram_tensor("wdir", [128, HT * KSH], f8e3,
                          kind="ExternalInput")
    wq8 = nc.dram_tensor("wq8", [128, (NT - HT) * KSH], mybir.dt.int8,
                         kind="ExternalInput")
    xt = nc.dram_tensor("xt", [NT // 8, 128, 8, BT], bf16,
                        kind="ExternalInput")
    srep = nc.dram_tensor("srep", [128, KSH], bf16, kind="ExternalInput")
    xgt = nc.dram_tensor("xgt", [NG + 1, BT], bf16, kind="ExternalInput")
    ct = nc.dram_tensor("ct", [NG + 1, KSH], bf16, kind="ExternalInput")
    out = nc.dram_tensor("out", [BT, KSH], bf16, kind="ExternalOutput")

    XCH = 8                   # xt tiles per DMA chunk
    NXC = NT // XCH           # 8 chunks
    RLOOK = 6                 # W-run DMA lookahead (runs)
    DQ = 10                   # dequant (DVE) lookahead over PE (tiles)

    with tile.TileContext(nc) as tc:
        with (
            tc.tile_pool(name="const", bufs=1) as cpool,
            tc.tile_pool(name="wd", bufs=4) as wdpool,
            tc.tile_pool(name="wq", bufs=4) as wqpool,
            tc.tile_pool(name="ws", bufs=12) as wspool,
            tc.tile_pool(name="psum", bufs=1, space="PSUM") as psum,
            tc.tile_pool(name="outp", bufs=1) as opool,
        ):
            xt_sb = cpool.tile([128, NT, BT], bf16, tag="xt")
            srep_sb = cpool.tile([128, KSH], bf16, tag="srep")
            xg_sb = cpool.tile([NG + 1, BT], bf16, tag="xg")
            ct_sb = cpool.tile([NG + 1, KSH], bf16, tag="ct")
            warm_sb = cpool.tile([128, 128], bf16, tag="warm")
            warm_ps = psum.tile([128, 128], mybir.dt.float32, tag="wps",
                                name="wps")

            # ---- PE warm-up: only dep is one DVE memset, so these run
            # during the head DMA wait and un-throttle the HAM clock.
            nc.vector.memset(warm_sb[:], 0.0)
            for _ in range(WARM):
                nc.tensor.matmul(warm_ps[:], warm_sb[:], warm_sb[:],
                                 start=True, stop=True)

            def load_xt_chunk(c, lo=0):
                nc.scalar.dma_start(
                    xt_sb[:, c * XCH + lo:(c + 1) * XCH, :],
                    xt[c, :, lo:XCH, :])

            run_tiles = {}

            def fetch_run(r):
                kind, tiles = RUNS[r]
                nt = len(tiles)
                off = WOFF[r] * KSH
                if kind == "dir":
                    w = wdpool.tile([128, 4 * KSH], f8e3, tag="wd", name="wd")
                    nc.sync.dma_start(w[:, :nt * KSH],
                                      wdir[:, off:off + nt * KSH])
                else:
                    w = wqpool.tile([128, 6 * KSH], mybir.dt.int8, tag="wq",
                                    name="wq")
                    nc.sync.dma_start(w[:, :nt * KSH],
                                      wq8[:, off:off + nt * KSH])
                run_tiles[r] = w

            ws_ready = {}

            def dequant(t):
                # fp8 tiles stream straight to the PE; int8 get one DVE mul
                r, i = RUN_OF[t]
                w = run_tiles[r]
                sl = w[:, i * KSH:(i + 1) * KSH]
                if RUNS[r][0] == "dir":
                    ws_ready[t] = sl
                else:
                    ws = wspool.tile([128, KSH], bf16, tag="ws", name="ws")
                    nc.vector.tensor_mul(ws[:], sl, srep_sb[:])
                    ws_ready[t] = ws[:]

            # head: W runs start immediately on SP; x/scales on ACT queue.
            # xt head split small->large so first tiles land progressively.
            nc.scalar.dma_start(xt_sb[:, 0:2, :], xt[0, :, 0:2, :])
            fetch_run(0)
            nc.scalar.dma_start(xt_sb[:, 2:5, :], xt[0, :, 2:5, :])
            nc.scalar.dma_start(srep_sb[:], srep[:])
            nc.scalar.dma_start(xt_sb[:, 5:8, :], xt[0, :, 5:8, :])
            load_xt_chunk(1)
            for r in range(1, RLOOK):
                fetch_run(r)
            for t in range(DQ):
                dequant(t)

            accs = [psum.tile([128, 512], mybir.dt.float32, tag=f"acc{b}{c}",
                              name=f"acc{b}{c}")
                    for b in range(2) for c in range(2)]

            nc.scalar.dma_start(xg_sb[:], xgt[:])
            nc.scalar.dma_start(ct_sb[:], ct[:])

            out_sb = opool.tile([128, 2, KSH], bf16, tag="o")
            out_v = out.ap().rearrange("(b p) k -> p b k", p=128)

            def evac_copy(b, c):
                # split PSUM evacuation: ACT copies b=0 chunks, DVE b=1
                sl = (slice(None), b, slice(c * 512, (c + 1) * 512))
                if b == 0:
                    nc.scalar.copy(out_sb[sl], accs[b * 2 + c][:])
                else:
                    nc.vector.tensor_copy(out_sb[sl], accs[b * 2 + c][:])

            for t in range(NT):
                r, i = RUN_OF[t]
                if i == 0 and r + RLOOK < len(RUNS):
                    fetch_run(r + RLOOK)
                if t + DQ < NT:
                    dequant(t + DQ)
                if t % XCH == 0 and t // XCH + 2 < NXC:
                    load_xt_chunk(t // XCH + 2)
                if t == 32:
                    # affine term + bias mid-stream: out += Xg2[bt,g] @ cT[g,k]
                    for b in range(2):
                        for c in range(2):
                            nc.tensor.matmul(
                                accs[b * 2 + c][:],
                                xg_sb[:, b * 128:(b + 1) * 128],
                                ct_sb[:, c * 512:(c + 1) * 512],
                                start=False, stop=False,
                            )
                ws = ws_ready.pop(t)
                for b in range(2):
                    for c in range(2):
                        nc.tensor.matmul(
                            accs[b * 2 + c][:],
                            xt_sb[:, t, b * 128:(b + 1) * 128],
                            ws[:, c * 512:(c + 1) * 512],
                            start=(t == 0), stop=(t == NT - 1),
                        )
                        if t == NT - 1:
                            evac_copy(b, c)

            # one out DMA per bt-half: sync for b=0, scalar for b=1
            nc.sync.dma_start(out_v[:, 0, :], out_sb[:, 0, :])
            nc.scalar.dma_start(out_v[:, 1, :], out_sb[:, 1, :])

    nc.compile()
    return nc


def _get_nc():
    global _NC_CACHE
    if _NC_CACHE is None:
        _NC_CACHE = _build()
    return _NC_CACHE


def _perm_index():
    # n_of[t, p] = original contraction index held by tile t, partition p
    t = np.arange(NT)[:, None]
    p = np.arange(128)[None, :]
    return (p // 2) * GROUP_SIZE + 2 * t + (p % 2)      # [NT, 128]


def _prep_in_maps(x, W_q, scales, zeros, mu1, mu2, bias):
    import ml_dtypes
    bf16 = ml_dtypes.bfloat16
    f8e3 = ml_dtypes.float8_e3m4
    x2 = np.asarray(x, dtype=np.float32).reshape(BT, N)
    mu1 = np.asarray(mu1, dtype=np.float32)
    mu2 = np.asarray(mu2, dtype=np.float32)
    bias = np.asarray(bias, dtype=np.float32)
    sc = np.asarray(scales, dtype=np.float32)[:, :, 0]   # [K, NG]
    zr = np.asarray(zeros, dtype=np.float32)[:, :, 0]    # [K, NG]
    W_q = np.asarray(W_q)

    n_of = _perm_index()                                  # [NT, 128]

    xp = x2 * mu1[None, :]                                # x' [BT, N]
    # [NXC=8, 128, XCH=8, BT]: partition-major per chunk
    xt_h = np.ascontiguousarray(
        xp.T[n_of.reshape(-1)].reshape(NT // 8, 8, 128, BT)
        .transpose(0, 2, 1, 3)).astype(bf16)
    Xg = xp.reshape(BT, NG, GROUP_SIZE).sum(axis=2)       # [BT, NG]
    xgt_h = np.concatenate(
        [np.ascontiguousarray(Xg.T), np.ones((1, BT), np.float32)],
        axis=0).astype(bf16)                              # [NG+1, BT]

    a = sc * mu2[:, None]                                 # [K, NG]
    a2 = 0.5 * a                                          # folded /2
    cmat = a * (7.5 - zr)                                 # centered affine
    g_of_p = np.arange(128) // 2                          # [128]
    Tq = (2 * W_q - 15).astype(np.float32)                # odd ints [K,N]

    dir_tiles = [t for t in range(NT) if IS_DIR[t]]
    i8_tiles = [t for t in range(NT) if not IS_DIR[t]]

    in_maps = []
    for i in range(NCORES):
        ksl = slice(i * KSH, (i + 1) * KSH)
        # [NT, 128, KSH]: tile-major, interleaved rows
        tq_perm = Tq[ksl, :].T[n_of.reshape(-1)].reshape(NT, 128, KSH)
        a2rep = a2[ksl, :].T[g_of_p, :].astype(np.float32)   # [128, KSH]
        srep_h = np.ascontiguousarray(a2rep).astype(bf16)
        # flat [128, HT*KSH] fp8 / [128, 36*KSH] int8, tiles in RUN order
        wdir_h = np.zeros((128, HT * KSH), dtype=f8e3)
        wq8_h = np.zeros((128, (NT - HT) * KSH), dtype=np.int8)
        for r, (kind, tiles) in enumerate(RUNS):
            for j, t in enumerate(tiles):
                o = (WOFF[r] + j) * KSH
                if kind == "dir":
                    wdir_h[:, o:o + KSH] = (tq_perm[t] * a2rep).astype(f8e3)
                else:
                    wq8_h[:, o:o + KSH] = tq_perm[t].astype(np.int8)
        ct_h = np.concatenate(
            [np.ascontiguousarray(cmat[ksl, :].T),
             bias[None, ksl]], axis=0).astype(bf16)       # [NG+1, KSH]
        in_maps.append({"wdir": wdir_h, "wq8": wq8_h, "xt": xt_h,
                        "srep": srep_h, "xgt": xgt_h, "ct": ct_h})
    return in_maps


def _run(inputs, trace=False):
    from concourse import bass_utils
    nc = _get_nc()
    in_maps = _prep_in_maps(**inputs)
    res = bass_utils.run_bass_kernel_spmd(
        nc, in_maps, core_ids=list(range(NCORES)), trace=trace)
    out = np.concatenate([res.results[i]["out"] for i in range(NCORES)],
                         axis=1)                          # [BT, K]
    return out.reshape(B, T, K).astype(np.float32), res


def kernel(**inputs) -> np.ndarray:
    out, _ = _run(inputs, trace=False)
    return out


def kernel_traced(**inputs):
    out, res = _run(inputs, trace=True)
    return out, res
